# revision 1
# baseline (speedup 1.0000x reference)
"""Trainium2 Bass kernel for the note/wiki 3-way contraction + gate MLP.

Math (per note n):
    e[n]    = (wikivec * notevec[n]) @ W_emb.T + b_emb          # (C, K)
    attn[n] = sigmoid(e[n] @ W_att.T + b_att)                   # (C, K)
    s[n]    = sum_k attn[n]*e[n]*W_out[0,k] + b_out             # (C,)

Sharding: data-parallel over the 16 notes -> 2 notes per core on 8 cores.
wikivec / W_emb are replicated (pre-transposed, zero-padded to 10112 = 79*128
along the contraction axis, cast to bf16 on the host so the per-core HBM->SBUF
stream is ~10 MB and hides under the PE work).

Device layout (all v-major so the contraction dim sits on partitions):
  phase 1: for each of 79 v-tiles, scale wikivec^T[v,:] by notevec[n,v]
           (per-partition scalar; note0 on DVE, note1 on ACT) into one
           [128, 512] bf16 moving tile, then 2 matmuls (k-halves) accumulate
           e^T[k, (note,c)] into two PSUM banks over all 79 v-tiles.
  phase 2: bias via ACT Identity, bf16 copy, 4 matmuls for attn logits,
           sigmoid, gate, W_out contraction, + b_out, DMA out s [1, 512].
"""

import sys

if "/opt/trn_rl_repo" not in sys.path:
    sys.path.insert(0, "/opt/trn_rl_repo")

import numpy as np
import ml_dtypes

import concourse.bass as bass
import concourse.mybir as mybir
import concourse.tile as tile
from concourse import bacc
from concourse.bass_utils import run_bass_kernel_spmd

N_CORES = 8
N, C, V, K = 16, 256, 10000, 256
J = 79  # number of 128-row v-tiles (V padded to 10112)
BLK = 8  # v-tiles per DMA block (DMA-issue on the Sync queue is ~700ns/op)
J2 = 80  # J padded to a multiple of BLK (pad tile is all-zero)
NB = J2 // BLK
VP = J * 128
NLOC = N // N_CORES  # notes per core

F32 = mybir.dt.float32
BF16 = mybir.dt.bfloat16
BF16_NP = ml_dtypes.bfloat16

_NC_CACHE = {}


def _build_nc():
    nc = bacc.Bacc(None, target_bir_lowering=False)

    wikiT = nc.declare_dram_parameter("wikiT", [NB, 128, BLK * C], BF16, isOutput=False)
    wembT = nc.declare_dram_parameter("wembT", [NB, 128, BLK * K], BF16, isOutput=False)
    scales = nc.declare_dram_parameter("scales", [128, NLOC * J2], F32, isOutput=False)
    watT = nc.declare_dram_parameter("watT", [2, 128, K], BF16, isOutput=False)
    woutT = nc.declare_dram_parameter("woutT", [128, 2], F32, isOutput=False)
    bemb = nc.declare_dram_parameter("bemb", [128, 2], F32, isOutput=False)
    batt = nc.declare_dram_parameter("batt", [128, 2], F32, isOutput=False)
    bout = nc.declare_dram_parameter("bout", [1, 1], F32, isOutput=False)
    s_out = nc.declare_dram_parameter("s_out", [1, NLOC * C], F32, isOutput=True)

    NC2 = NLOC * C  # 512: (note, c) column block

    with tile.TileContext(nc) as tc:
        with (
            tc.tile_pool(name="const", bufs=1) as constp,
            tc.tile_pool(name="wt", bufs=4) as wtp,
            tc.tile_pool(name="et", bufs=4) as etp,
            tc.tile_pool(name="mov", bufs=4) as movp,
            tc.tile_pool(name="post", bufs=1) as postp,
            tc.tile_pool(name="psum", bufs=1, space="PSUM") as psp,
        ):
            sc = constp.tile([128, NLOC * J2], F32)
            nc.sync.dma_start(sc[:], scales[:])
            wat = constp.tile([128, 2 * K], BF16)
            nc.sync.dma_start(wat[:, 0:K], watT[0])
            nc.sync.dma_start(wat[:, K : 2 * K], watT[1])
            wout = constp.tile([128, 2], F32)
            nc.sync.dma_start(wout[:], woutT[:])
            be = constp.tile([128, 2], F32)
            nc.sync.dma_start(be[:], bemb[:])
            ba = constp.tile([128, 2], F32)
            nc.sync.dma_start(ba[:], batt[:])
            bo = constp.tile([1, 1], F32)
            nc.sync.dma_start(bo[:], bout[:])

            # Warmup reads: the activation engine only supports a single
            # sync-wait per instruction, so let ACT/DVE observe the constant
            # DMA semaphore lanes up front, one lane per tiny instruction.
            warm0 = constp.tile([128, 1], F32)
            nc.scalar.copy(warm0[:], be[:, 0:1])
            warm1 = constp.tile([128, 1], F32)
            nc.scalar.copy(warm1[:], ba[:, 0:1])
            warm2 = constp.tile([1, 1], F32)
            nc.scalar.copy(warm2[:], bo[:])
            warmd = constp.tile([128, 1], F32)
            nc.vector.tensor_copy(warmd[:], sc[:, 0:1])

            # e^T accumulators: [k-half 128, (note,c) 512] fp32, one bank each
            e_ps = [
                psp.tile([128, NC2], F32, name=f"e_ps{m}", tag=f"e_ps{m}")
                for m in range(2)
            ]

            for b in range(NB):
                wt = wtp.tile([128, BLK * C], BF16)
                nc.sync.dma_start(wt[:], wikiT[b])
                et = etp.tile([128, BLK * K], BF16)
                nc.sync.dma_start(et[:], wembT[b])
                for jj in range(BLK):
                    j = b * BLK + jj
                    wts = wt[:, jj * C : (jj + 1) * C]
                    mov = movp.tile([128, NC2], BF16)
                    # note0 on DVE, note1 on ACT (GpSimd shares SBUF ports
                    # with DVE and wrecks both when run concurrently)
                    nc.vector.tensor_scalar_mul(mov[:, 0:C], wts, sc[:, j : j + 1])
                    nc.scalar.mul(
                        mov[:, C : 2 * C], wts, mul=sc[:, J2 + j : J2 + j + 1]
                    )
                    st, sp = (j == 0), (j == J2 - 1)
                    for m in range(2):
                        nc.tensor.matmul(
                            e_ps[m][:],
                            et[:, jj * K + m * 128 : jj * K + (m + 1) * 128],
                            mov[:],
                            start=st,
                            stop=sp,
                        )

            # ---- phase 2: bias, attn logits, sigmoid, gate, W_out ----
            ef = []
            eb = []
            for m in range(2):
                ef_m = postp.tile([128, NC2], F32, tag=f"ef{m}")
                nc.scalar.activation(
                    ef_m[:],
                    e_ps[m][:],
                    mybir.ActivationFunctionType.Identity,
                    bias=be[:, m : m + 1],
                    scale=1.0,
                )
                eb_m = postp.tile([128, NC2], BF16, tag=f"eb{m}")
                nc.vector.tensor_copy(eb_m[:], ef_m[:])
                ef.append(ef_m)
                eb.append(eb_m)

            a_ps = [
                psp.tile([128, NC2], F32, name=f"a_ps{jm}", tag=f"a_ps{jm}")
                for jm in range(2)
            ]
            for kt in range(2):
                for jm in range(2):
                    nc.tensor.matmul(
                        a_ps[jm][:],
                        wat[:, kt * K + jm * 128 : kt * K + (jm + 1) * 128],
                        eb[kt][:],
                        start=(kt == 0),
                        stop=(kt == 1),
                    )

            v = []
            for jm in range(2):
                atn = postp.tile([128, NC2], F32, tag=f"atn{jm}")
                nc.scalar.activation(
                    atn[:],
                    a_ps[jm][:],
                    mybir.ActivationFunctionType.Sigmoid,
                    bias=ba[:, jm : jm + 1],
                    scale=1.0,
                )
                v_jm = postp.tile([128, NC2], F32, tag=f"v{jm}")
                nc.vector.tensor_mul(v_jm[:], atn[:], ef[jm][:])
                v.append(v_jm)

            s_ps = psp.tile([1, NC2], F32, tag="s_ps")
            for kt in range(2):
                nc.tensor.matmul(
                    s_ps[:],
                    wout[:, kt : kt + 1],
                    v[kt][:],
                    start=(kt == 0),
                    stop=(kt == 1),
                )
            s_sb = postp.tile([1, NC2], F32, tag="s_sb")
            nc.scalar.activation(
                s_sb[:],
                s_ps[:],
                mybir.ActivationFunctionType.Identity,
                bias=bo[0:1, 0:1],
                scale=1.0,
            )
            nc.sync.dma_start(s_out[:], s_sb[:])

    nc.compile()
    return nc


def _get_nc():
    if "nc" not in _NC_CACHE:
        _NC_CACHE["nc"] = _build_nc()
    return _NC_CACHE["nc"]


def _pad_T_tile(a):
    """(rows, V) -> zero-padded (NB, 128, BLK*rows) transposed block tiles,
    bf16; block b col jj*rows+c holds a.T[(b*BLK+jj)*128 + p, c]."""
    rows = a.shape[0]
    out = np.zeros((J2 * 128, rows), np.float32)
    out[:V] = a.T
    out = out.reshape(NB, BLK, 128, rows).transpose(0, 2, 1, 3)
    return np.ascontiguousarray(out.reshape(NB, 128, BLK * rows)).astype(BF16_NP)


def prep_inputs(notevec, wikivec, W_emb, b_emb, W_att, b_att, W_out, b_out):
    wikiT = _pad_T_tile(np.asarray(wikivec, np.float32))
    wembT = _pad_T_tile(np.asarray(W_emb, np.float32))
    watT = np.ascontiguousarray(
        np.asarray(W_att, np.float32).T.reshape(2, 128, K)
    ).astype(BF16_NP)
    woutT = np.ascontiguousarray(
        np.asarray(W_out, np.float32)[0].reshape(2, 128).T
    )
    bemb = np.ascontiguousarray(np.asarray(b_emb, np.float32).reshape(2, 128).T)
    batt = np.ascontiguousarray(np.asarray(b_att, np.float32).reshape(2, 128).T)
    bout = np.asarray(b_out, np.float32).reshape(1, 1)

    nv = np.zeros((N, J2 * 128), np.float32)
    nv[:, :V] = np.asarray(notevec, np.float32)
    in_maps = []
    for i in range(N_CORES):
        # scales[p, l*J2 + j] = notevec[2i+l, j*128+p]
        sc = np.ascontiguousarray(
            nv[i * NLOC : (i + 1) * NLOC].reshape(NLOC, J2, 128).transpose(2, 0, 1)
        ).reshape(128, NLOC * J2)
        in_maps.append(
            {
                "wikiT": wikiT,
                "wembT": wembT,
                "scales": np.ascontiguousarray(sc),
                "watT": watT,
                "woutT": woutT,
                "bemb": bemb,
                "batt": batt,
                "bout": bout,
            }
        )
    return in_maps


def run(in_maps, **kw):
    nc = _get_nc()
    return run_bass_kernel_spmd(nc, in_maps, list(range(N_CORES)), **kw)


def kernel(notevec, wikivec, W_emb, b_emb, W_att, b_att, W_out, b_out):
    in_maps = prep_inputs(
        notevec, wikivec, W_emb, b_emb, W_att, b_att, W_out, b_out
    )
    res = run(in_maps)
    out = np.concatenate(
        [r["s_out"].reshape(NLOC, C) for r in res.results], axis=0
    )
    return out.astype(np.float32)



# revision 8
# speedup vs baseline: 1.1206x; 1.1206x over previous
"""Trainium2 Bass kernel for the note/wiki 3-way contraction + gate MLP.

Math (per note n):
    e[n]    = (wikivec * notevec[n]) @ W_emb.T + b_emb          # (C, K)
    attn[n] = sigmoid(e[n] @ W_att.T + b_att)                   # (C, K)
    s[n]    = sum_k attn[n]*e[n]*W_out[0,k] + b_out             # (C,)

Sharding: 2D (note x code): 4 note-groups x 2 code-groups on 8 cores.
Core i handles notes [4*(i//2), 4*(i//2)+4) and codes [128*(i%2), ...+128).
This keeps the per-v-tile moving tile at [128, 4*128=512] columns (same PE
cost as pure data-parallel) while cutting the per-core HBM stream from
10.4 MB (wiki+wemb replicated) to 7.85 MB.

All phase-1 data rides ONE v-tile-major bf16 stream laid out host-side as
the exact SBUF image: per v-tile j, 388 columns = [noteT(4) | wikiT(128) |
wembT(256)].  Blocks of v-tiles are DMA'd with small blocks first so the
PE starts early; dummy matmuls on a memset tile warm the PE clock (HAM)
during the initial DMA wait, and a dummy sigmoid preloads the ACT table.

Phase 1 per v-tile: mov[128, l*128+c] = wikiT[v,c] * noteT[v,l] via 3
tensor_scalar ops on DVE + 1 on ACT, then 2 matmuls (k-halves) accumulate
e^T[k, (l,c)] into two PSUM banks over all 79 v-tiles.

Phase 2: eb = bf16(e + b_emb) on DVE, 4 matmuls for attn logits, sigmoid
(+b_att) on ACT, gate on DVE, W_out contraction, +b_out on DVE, DMA out.
"""

import sys

if "/opt/trn_rl_repo" not in sys.path:
    sys.path.insert(0, "/opt/trn_rl_repo")

import numpy as np
import ml_dtypes

import concourse.bass as bass
import concourse.mybir as mybir
import concourse.tile as tile
from concourse import bacc
from concourse.bass_utils import run_bass_kernel_spmd

N_CORES = 8
N, C, V, K = 16, 256, 10000, 256
J = 79  # number of 128-row v-tiles (V zero-padded to 10112)
NLOC = 4  # notes per core (4 note-groups)
CLOC = 128  # codes per core (2 code-groups)
NC2 = NLOC * CLOC  # 512 moving columns (note-major: col = l*128 + c)
COLS = CLOC + K  # 384 stream columns per v-tile: [wikiT(128) | wembT(256)]
BLOCKS = [2, 2, 4, 8, 8, 8, 8, 8, 8, 8, 8, 7]  # v-tiles per DMA block
assert sum(BLOCKS) == J
N_WARM_MM = 8  # dummy PE matmuls to lift the HAM clock gate during DMA wait

F32 = mybir.dt.float32
BF16 = mybir.dt.bfloat16
BF16_NP = ml_dtypes.bfloat16

_NC_CACHE = {}


def _build_nc():
    nc = bacc.Bacc(None, target_bir_lowering=False)

    stream = nc.declare_dram_parameter("stream", [128, J * COLS], BF16, isOutput=False)
    scales = nc.declare_dram_parameter("scales", [128, NLOC * J], F32, isOutput=False)
    watx = nc.declare_dram_parameter("watx", [128, 2 * K + 2], BF16, isOutput=False)
    cf = nc.declare_dram_parameter("cf", [128, 6], F32, isOutput=False)
    s_out = nc.declare_dram_parameter("s_out", [1, NC2], F32, isOutput=True)

    SIG = mybir.ActivationFunctionType.Sigmoid

    with tile.TileContext(nc) as tc:
        with (
            tc.tile_pool(name="const", bufs=1) as constp,
            tc.tile_pool(name="st", bufs=4) as stp,
            tc.tile_pool(name="mov", bufs=4) as movp,
            tc.tile_pool(name="post", bufs=1) as postp,
            tc.tile_pool(name="psum", bufs=1, space="PSUM") as psp,
        ):
            # ---- warmups (no DMA deps): PE clock + ACT sigmoid table ----
            warm = constp.tile([128, 512], BF16)
            nc.vector.memset(warm[:], 0.5)
            warm_ps = psp.tile([128, 256], F32, tag="warm_ps")
            for _ in range(N_WARM_MM):
                nc.tensor.matmul(
                    warm_ps[:], warm[:, 0:128], warm[:, 0:256], start=True, stop=True
                )
            warm_sig = constp.tile([128, 1], F32)
            nc.scalar.activation(warm_sig[:], warm[:, 0:1], SIG, bias=0.0, scale=1.0)

            # ---- DMAs: stream blocks on the Sync ring, consts on ACT ring
            sc = constp.tile([128, NLOC * J], F32)
            nc.scalar.dma_start(sc[:], scales[:])
            cfs = constp.tile([128, 6], F32)
            nc.scalar.dma_start(cfs[:], cf[:])
            wat = constp.tile([128, 2 * K + 2], BF16)
            nc.scalar.dma_start(wat[:], watx[:])

            # let ACT/DVE observe the const-DMA semaphores early (single
            # sync-wait per ACT instruction), one tiny instruction each
            warm_s0 = constp.tile([128, 1], F32)
            nc.scalar.copy(warm_s0[:], sc[:, 0:1])
            warm_s1 = constp.tile([128, 1], F32)
            nc.vector.tensor_copy(warm_s1[:], sc[:, 1:2])
            warm_c0 = constp.tile([128, 1], F32)
            nc.scalar.copy(warm_c0[:], cfs[:, 0:1])
            warm_c1 = constp.tile([128, 1], F32)
            nc.vector.tensor_copy(warm_c1[:], cfs[:, 1:2])

            # e^T accumulators: [k-half 128, (l,c) 512] fp32, one bank each
            e_ps = [
                psp.tile([128, NC2], F32, name=f"e_ps{m}", tag=f"e_ps{m}")
                for m in range(2)
            ]

            j = 0
            off = 0
            for b, nb in enumerate(BLOCKS):
                st = stp.tile([128, nb * COLS], BF16, tag="st")
                nc.sync.dma_start(st[:], stream[:, off * COLS : (off + nb) * COLS])
                off += nb
                for jj in range(nb):
                    base = jj * COLS
                    wk = st[:, base : base + CLOC]
                    mov = movp.tile([128, NC2], BF16)
                    # notes 0-2 on DVE, note 3 on ACT
                    for l in range(3):
                        nc.vector.tensor_scalar_mul(
                            mov[:, l * CLOC : (l + 1) * CLOC],
                            wk,
                            sc[:, l * J + j : l * J + j + 1],
                        )
                    nc.scalar.mul(
                        mov[:, 3 * CLOC : 4 * CLOC],
                        wk,
                        mul=sc[:, 3 * J + j : 3 * J + j + 1],
                    )
                    st_, sp_ = (j == 0), (j == J - 1)
                    wb = base + CLOC
                    for m in range(2):
                        nc.tensor.matmul(
                            e_ps[m][:],
                            st[:, wb + m * 128 : wb + (m + 1) * 128],
                            mov[:],
                            start=st_,
                            stop=sp_,
                        )
                    j += 1

            # ---- phase 2 ----
            eb = []
            for m in range(2):
                eb_m = postp.tile([128, NC2], BF16, tag=f"eb{m}")
                nc.vector.tensor_scalar_add(eb_m[:], e_ps[m][:], cfs[:, m : m + 1])
                eb.append(eb_m)

            a_ps = [
                psp.tile([128, NC2], F32, name=f"a_ps{jm}", tag=f"a_ps{jm}")
                for jm in range(2)
            ]
            for kt in range(2):
                for jm in range(2):
                    nc.tensor.matmul(
                        a_ps[jm][:],
                        wat[:, kt * K + jm * 128 : kt * K + (jm + 1) * 128],
                        eb[kt][:],
                        start=(kt == 0),
                        stop=(kt == 1),
                    )

            v = []
            for jm in range(2):
                atn = postp.tile([128, NC2], BF16, tag=f"atn{jm}")
                nc.scalar.activation(
                    atn[:], a_ps[jm][:], SIG, bias=cfs[:, 2 + jm : 3 + jm], scale=1.0
                )
                v_jm = postp.tile([128, NC2], BF16, tag=f"v{jm}")
                nc.vector.tensor_mul(v_jm[:], atn[:], eb[jm][:])
                v.append(v_jm)

            s_ps = psp.tile([1, NC2], F32, tag="s_ps")
            for kt in range(2):
                nc.tensor.matmul(
                    s_ps[:],
                    wat[:, 2 * K + kt : 2 * K + kt + 1],
                    v[kt][:],
                    start=(kt == 0),
                    stop=(kt == 1),
                )
            s_sb = postp.tile([1, NC2], F32, tag="s_sb")
            nc.vector.tensor_scalar_add(s_sb[:], s_ps[:], cfs[0:1, 4:5])
            nc.sync.dma_start(s_out[:], s_sb[:])

    nc.compile()
    return nc


def _get_nc():
    if "nc" not in _NC_CACHE:
        _NC_CACHE["nc"] = _build_nc()
    return _NC_CACHE["nc"]


def prep_inputs(notevec, wikivec, W_emb, b_emb, W_att, b_att, W_out, b_out):
    notevec = np.asarray(notevec, np.float32)
    wikivec = np.asarray(wikivec, np.float32)
    W_emb = np.asarray(W_emb, np.float32)

    # shared template: v-major [J*128, COLS] with the wemb part filled
    tmpl = np.zeros((J * 128, COLS), BF16_NP)
    tmpl[:V, CLOC:] = W_emb.T.astype(BF16_NP)
    wikiT = wikivec.T.astype(BF16_NP)  # (V, 256)

    # scales[p, l*J + j] = notevec[note l of core, j*128 + p], f32
    nv = np.zeros((N, J * 128), np.float32)
    nv[:, :V] = notevec

    # attn stationary [kp, kt*256 + j] plus W_out columns [kp, kt]
    watk = np.ascontiguousarray(
        np.asarray(W_att, np.float32).T.reshape(2, 128, K).transpose(1, 0, 2)
    ).reshape(128, 2 * K)
    wo = np.asarray(W_out, np.float32)[0].reshape(2, 128).T
    watx = np.concatenate([watk, wo], axis=1).astype(BF16_NP)

    cfh = np.zeros((128, 6), np.float32)
    cfh[:, 0:2] = np.asarray(b_emb, np.float32).reshape(2, 128).T
    cfh[:, 2:4] = np.asarray(b_att, np.float32).reshape(2, 128).T
    cfh[:, 4] = np.asarray(b_out, np.float32)[0]

    in_maps = []
    for i in range(N_CORES):
        a, bb = divmod(i, 2)
        img = tmpl.copy()
        img[:V, 0:CLOC] = wikiT[:, bb * CLOC : (bb + 1) * CLOC]
        strm = np.ascontiguousarray(
            img.reshape(J, 128, COLS).transpose(1, 0, 2)
        ).reshape(128, J * COLS)
        sch = np.ascontiguousarray(
            nv[a * NLOC : (a + 1) * NLOC].reshape(NLOC, J, 128).transpose(2, 0, 1)
        ).reshape(128, NLOC * J)
        in_maps.append(
            {
                "stream": strm,
                "scales": sch,
                "watx": watx,
                "cf": cfh,
            }
        )
    return in_maps


def run(in_maps, **kw):
    nc = _get_nc()
    return run_bass_kernel_spmd(nc, in_maps, list(range(N_CORES)), **kw)


def gather(results):
    out = np.zeros((N, C), np.float32)
    for i, r in enumerate(results):
        a, bb = divmod(i, 2)
        out[a * NLOC : (a + 1) * NLOC, bb * CLOC : (bb + 1) * CLOC] = (
            r["s_out"].reshape(NLOC, CLOC)
        )
    return out


def kernel(notevec, wikivec, W_emb, b_emb, W_att, b_att, W_out, b_out):
    in_maps = prep_inputs(
        notevec, wikivec, W_emb, b_emb, W_att, b_att, W_out, b_out
    )
    res = run(in_maps)
    return gather(res.results)


# revision 10
# speedup vs baseline: 1.2281x; 1.0959x over previous
"""Trainium2 Bass kernel for the note/wiki 3-way contraction + gate MLP.

Math (per note n):
    e[n]    = (wikivec * notevec[n]) @ W_emb.T + b_emb          # (C, K)
    attn[n] = sigmoid(e[n] @ W_att.T + b_att)                   # (C, K)
    s[n]    = sum_k attn[n]*e[n]*W_out[0,k] + b_out             # (C,)

Sharding: data-parallel over the 16 notes -> 2 notes per core on 8 cores
(wikivec / W_emb replicated).  All device data rides ONE v-tile-major bf16
stream laid out host-side as the exact SBUF image: per v-tile j, 512
columns = [wikiT(256) | wembT(256)].  The f32 note scales are bitcast into
the first block and the phase-2 constants (W_att^T, W_out, biases) into
the last block, so there are no extra DMAs / semaphore lanes.

Phase 1 per v-tile: mov[128, l*256+c] = wikiT[v,c] * note_l[v] via two
[128,256] tensor_scalar ops, BOTH on DVE (ACT's ~350ns fixed op overhead
made it the production bottleneck; DVE runs these at ~197ns in 4x mode,
so 2 ops = 394ns < the 432ns the PE needs per v-tile).  Then 2 matmuls
(k-halves) accumulate e^T[k, (l,c)] into two PSUM banks over 79 v-tiles.

Dummy matmuls on a memset tile bridge the ~5us DMA latency of the first
block AND hold the PE clock-gate (HAM) at full rate; a dummy sigmoid
preloads the ACT function table so the phase-2 sigmoid doesn't stall.

Phase 2: eb = bf16(e + b_emb) on DVE, 4 matmuls for attn logits, sigmoid
(+b_att) on ACT, gate on DVE (bf16), W_out contraction, +b_out on DVE,
DMA out s [1, 512].
"""

import sys

if "/opt/trn_rl_repo" not in sys.path:
    sys.path.insert(0, "/opt/trn_rl_repo")

import numpy as np
import ml_dtypes

import concourse.bass as bass
import concourse.mybir as mybir
import concourse.tile as tile
from concourse import bacc
from concourse.bass_utils import run_bass_kernel_spmd

N_CORES = 8
N, C, V, K = 16, 256, 10000, 256
J = 79  # number of 128-row v-tiles (V zero-padded to 10112)
NLOC = 2  # notes per core
CLOC = C  # codes per core (replicated)
NC2 = NLOC * C  # 512 moving columns (col = l*256 + c)
COLS = C + K  # 512 stream columns per v-tile: [wikiT(256) | wembT(256)]
SCW = NLOC * J  # 158 f32 scale words per partition
WATW = 2 * K + 2  # attn stationary + W_out columns (bf16)
CFW = 6  # f32 const words: bemb(2) batt(2) bout(1) pad(1)
BLOCKS = [2, 2, 2, 4, 8, 8, 8, 8, 8, 8, 8, 8, 5]  # v-tiles per DMA block
assert sum(BLOCKS) == J
N_WARM_MM = 16  # dummy PE matmuls bridging the first-block DMA latency

F32 = mybir.dt.float32
BF16 = mybir.dt.bfloat16
BF16_NP = ml_dtypes.bfloat16

_NC_CACHE = {}


def _build_nc():
    nc = bacc.Bacc(None, target_bir_lowering=False)

    # stream columns: [sc bitcast (2*SCW) | blocks of v-tiles | watx | cf]
    TOTC = 2 * SCW + J * COLS + WATW + 2 * CFW
    stream = nc.declare_dram_parameter("stream", [128, TOTC], BF16, isOutput=False)
    s_out = nc.declare_dram_parameter("s_out", [1, NC2], F32, isOutput=True)

    SIG = mybir.ActivationFunctionType.Sigmoid

    with tile.TileContext(nc) as tc:
        with (
            tc.tile_pool(name="const", bufs=1) as constp,
            tc.tile_pool(name="st", bufs=4) as stp,
            tc.tile_pool(name="mov", bufs=4) as movp,
            tc.tile_pool(name="post", bufs=1) as postp,
            tc.tile_pool(name="psum", bufs=1, space="PSUM") as psp,
        ):
            # ---- warmups (no DMA deps): PE clock gate + ACT sigmoid table
            warm = constp.tile([128, 256], BF16)
            nc.vector.memset(warm[:], 0.5)
            warm_ps = psp.tile([128, 256], F32, tag="warm_ps")
            for _ in range(N_WARM_MM):
                nc.tensor.matmul(
                    warm_ps[:], warm[:, 0:128], warm[:], start=True, stop=True
                )
            warm_sig = constp.tile([128, 1], F32)
            nc.scalar.activation(warm_sig[:], warm[:, 0:1], SIG, bias=0.0, scale=1.0)

            # e^T accumulators: [k-half 128, (l,c) 512] fp32, one bank each
            e_ps = [
                psp.tile([128, NC2], F32, name=f"e_ps{m}", tag=f"e_ps{m}")
                for m in range(2)
            ]

            sc = None  # f32 view of the note scales (in block 0)
            wat = None  # phase-2 constants (in the last block)
            cfs = None
            j = 0
            off = 0
            for b, nb in enumerate(BLOCKS):
                w = nb * COLS
                pre = 2 * SCW if b == 0 else 0
                post_w = WATW + 2 * CFW if b == len(BLOCKS) - 1 else 0
                # block 0 (note scales) and the last block (phase-2 consts)
                # must outlive the 4-buffer rotation
                if b == 0:
                    tag, bufs = "st0", 1
                elif b == len(BLOCKS) - 1:
                    tag, bufs = "stZ", 1
                else:
                    tag, bufs = "st", None
                st = stp.tile([128, pre + w + post_w], BF16, tag=tag, bufs=bufs)
                nc.sync.dma_start(st[:], stream[:, off : off + pre + w + post_w])
                off += pre + w + post_w
                if b == 0:
                    sc = st[:, 0 : 2 * SCW].bitcast(F32)
                if b == len(BLOCKS) - 1:
                    wat = st[:, pre + w : pre + w + WATW]
                    cfs = st[:, pre + w + WATW :].bitcast(F32)
                for jj in range(nb):
                    base = pre + jj * COLS
                    wk = st[:, base : base + C]
                    mov = movp.tile([128, NC2], BF16)
                    for l in range(NLOC):
                        nc.vector.tensor_scalar_mul(
                            mov[:, l * C : (l + 1) * C],
                            wk,
                            sc[:, l * J + j : l * J + j + 1],
                        )
                    st_, sp_ = (j == 0), (j == J - 1)
                    for m in range(2):
                        nc.tensor.matmul(
                            e_ps[m][:],
                            st[:, base + C + m * 128 : base + C + (m + 1) * 128],
                            mov[:],
                            start=st_,
                            stop=sp_,
                        )
                    j += 1

            # ---- phase 2 ----
            eb = []
            for m in range(2):
                eb_m = postp.tile([128, NC2], BF16, tag=f"eb{m}")
                nc.vector.tensor_scalar_add(eb_m[:], e_ps[m][:], cfs[:, m : m + 1])
                eb.append(eb_m)

            a_ps = [
                psp.tile([128, NC2], F32, name=f"a_ps{jm}", tag=f"a_ps{jm}")
                for jm in range(2)
            ]
            for kt in range(2):
                for jm in range(2):
                    nc.tensor.matmul(
                        a_ps[jm][:],
                        wat[:, kt * K + jm * 128 : kt * K + (jm + 1) * 128],
                        eb[kt][:],
                        start=(kt == 0),
                        stop=(kt == 1),
                    )

            v = []
            for jm in range(2):
                atn = postp.tile([128, NC2], BF16, tag=f"atn{jm}")
                nc.scalar.activation(
                    atn[:], a_ps[jm][:], SIG, bias=cfs[:, 2 + jm : 3 + jm], scale=1.0
                )
                v_jm = postp.tile([128, NC2], BF16, tag=f"v{jm}")
                nc.vector.tensor_mul(v_jm[:], atn[:], eb[jm][:])
                v.append(v_jm)

            s_ps = psp.tile([1, NC2], F32, tag="s_ps")
            for kt in range(2):
                nc.tensor.matmul(
                    s_ps[:],
                    wat[:, 2 * K + kt : 2 * K + kt + 1],
                    v[kt][:],
                    start=(kt == 0),
                    stop=(kt == 1),
                )
            s_sb = postp.tile([1, NC2], F32, tag="s_sb")
            nc.vector.tensor_scalar_add(s_sb[:], s_ps[:], cfs[0:1, 4:5])
            nc.sync.dma_start(s_out[:], s_sb[:])

    nc.compile()
    return nc


def _get_nc():
    if "nc" not in _NC_CACHE:
        _NC_CACHE["nc"] = _build_nc()
    return _NC_CACHE["nc"]


def prep_inputs(notevec, wikivec, W_emb, b_emb, W_att, b_att, W_out, b_out):
    notevec = np.asarray(notevec, np.float32)
    wikivec = np.asarray(wikivec, np.float32)
    W_emb = np.asarray(W_emb, np.float32)

    # v-tile images, v-major [J*128, COLS]: [wikiT | wembT] (zero-padded v)
    img = np.zeros((J * 128, COLS), BF16_NP)
    img[:V, 0:C] = wikivec.T.astype(BF16_NP)
    img[:V, C:] = W_emb.T.astype(BF16_NP)
    tiles = np.ascontiguousarray(
        img.reshape(J, 128, COLS).transpose(1, 0, 2)
    ).reshape(128, J * COLS)

    # scales[p, l*J + j] = notevec[2i+l, j*128 + p], f32 viewed as bf16 pairs
    nv = np.zeros((N, J * 128), np.float32)
    nv[:, :V] = notevec

    # attn stationary [kp, kt*256 + jcol] plus W_out columns [kp, kt]
    watk = np.ascontiguousarray(
        np.asarray(W_att, np.float32).T.reshape(2, 128, K).transpose(1, 0, 2)
    ).reshape(128, 2 * K)
    wo = np.asarray(W_out, np.float32)[0].reshape(2, 128).T
    watx = np.concatenate([watk, wo], axis=1).astype(BF16_NP)

    cfh = np.zeros((128, CFW), np.float32)
    cfh[:, 0:2] = np.asarray(b_emb, np.float32).reshape(2, 128).T
    cfh[:, 2:4] = np.asarray(b_att, np.float32).reshape(2, 128).T
    cfh[:, 4] = np.asarray(b_out, np.float32)[0]
    cf_bf = cfh.view(BF16_NP)  # [128, 2*CFW] raw bytes

    in_maps = []
    for i in range(N_CORES):
        sch = np.ascontiguousarray(
            nv[i * NLOC : (i + 1) * NLOC].reshape(NLOC, J, 128).transpose(2, 0, 1)
        ).reshape(128, SCW)
        sc_bf = sch.view(BF16_NP)  # [128, 2*SCW] raw bytes
        strm = np.concatenate([sc_bf, tiles, watx, cf_bf], axis=1)
        in_maps.append({"stream": np.ascontiguousarray(strm)})
    return in_maps


def run(in_maps, **kw):
    nc = _get_nc()
    return run_bass_kernel_spmd(nc, in_maps, list(range(N_CORES)), **kw)


def gather(results):
    out = np.zeros((N, C), np.float32)
    for i, r in enumerate(results):
        out[i * NLOC : (i + 1) * NLOC, :] = r["s_out"].reshape(NLOC, C)
    return out


def kernel(notevec, wikivec, W_emb, b_emb, W_att, b_att, W_out, b_out):
    in_maps = prep_inputs(
        notevec, wikivec, W_emb, b_emb, W_att, b_att, W_out, b_out
    )
    res = run(in_maps)
    return gather(res.results)
